# revision 9
# baseline (speedup 1.0000x reference)
"""Trainium2 Bass kernel for a 2-layer dense GAT (nn_GAT_87144886436203).

Sharding: row-shard the N=4096 nodes across 8 NeuronCores (512 rows each).
Each core computes attention scores for its row block against all N columns,
with the contraction axis j kept on SBUF partitions.

Score pipeline, two interchangeable per-group paths over [128 j, 2048 i]
fp16 tiles:
  A-path (ACT):  u = Prelu(frep + g[j]) per chunk; p = Exp(u) (2048-wide)
  D-path (DVE):  exp(lrelu(f+g)) == max(e^f e^g, e^.2f e^.2g)  (exp monotone)
                 P1 = EfRep * Eg[j]; P2 = efRep * eg[j]  (4x tensor_scalar)
                 q  = max(P1, P2)
Masking uses min(q, adjM) with adjM in {+inf, 0} -- `min` runs in the
GPSIMD 0.6-efficiency class (vs 0.42 for mult), and q > 0 always.

The att matmul runs transposed: out[i_sub 128, 64] += pm[j, i_sub].T @ Wh[j, :]
(PE cost is proportional to OUT free size, so 64-wide beats 512-wide 8x),
with the row-sum accumulated by a second [128,1] ones matmul. The softmax
reciprocal then lands per-partition (cheap tensor_scalar, no broadcast
matmul). hcat is re-transposed to hcatT via 16 PE transposes for layer 2.
One small AllGather moves the per-core layer-2 node features between layers.

Engine assignment (A vs D path, and DVE vs Pool for the max/mask
tensor_tensor ops and prep copies) is tunable via the *_COUNT knobs,
balanced against the TimelineSim cost model.
"""

import numpy as np
import ml_dtypes

import concourse.bass as bass
import concourse.bacc as bacc
import concourse.tile as tile
import concourse.mybir as mybir
from concourse import masks
from concourse.bass_utils import run_bass_kernel_spmd

F16 = mybir.dt.float16
F32 = mybir.dt.float32
NPF16 = ml_dtypes.float16 if hasattr(ml_dtypes, "float16") else np.float16

NCORES = 8
N = 4096            # nodes
K = 512             # input feature dim (= NFEAT)
H = 8               # heads (layer 1)
D = 64              # per-head hidden (= NHID = NCLASS)
DALL = H * D        # 512
R = N // NCORES     # 512 rows per core
JC = N // 128       # 32 j-chunks
G = 4               # j-chunks per group (free dim 2048 for the big ops)
NG = JC // G        # 8 groups
AUG2 = D + 2        # 66: [Wh2 | ones | g2]
ALPHA = 0.2

N_UNITS = H + 1
NSLOTS = N_UNITS * NG     # 72 (unit, group) slots

# ---- engine-assignment knobs (tuned against TimelineSim) ----
A_COUNT = 30       # slots on the ACT path (rest: DVE separable path)
MAXP_COUNT = 22    # of the D-slots' max ops, how many go to Pool
MASKP_COUNT = 30   # of the 72 mask ops, how many go to Pool
PREPC_POOL = 10    # of the 32 whb prep copies, how many on Pool
PREPC_ACT = 10     # ... how many on ACT (rest DVE)
EPI_POOL = 18      # of 36 epilogue subtiles, t2/add ops on Pool
HCT_ACT = 8        # of 16 hcatT copies, how many on ACT (rest DVE)


def _bres(i, count, total):
    return (i * count) // total != ((i + 1) * count) // total


def _slot(unit, g):
    return unit * NG + g


def _is_A(unit, g):
    # lead units early-on prefer ACT anyway; simple prefix assignment with
    # layer-2 unit mixed in via bres for tail balance
    s = _slot(unit, g)
    if unit == H:
        return _bres(g, 3, NG)      # 3 of 8 layer-2 groups on ACT
    return s < A_COUNT


def _max_on_pool(unit, g):
    return _bres(_slot(unit, g), MAXP_COUNT, NSLOTS)


def _mask_on_pool(unit, g):
    return _bres(_slot(unit, g), MASKP_COUNT, NSLOTS)


_CACHE = {}


# --------------------------------------------------------------------------- #
# device program
# --------------------------------------------------------------------------- #

def _build(emulate_collective=False):
    nc = bacc.Bacc(
        "TRN2",
        target_bir_lowering=False,
        debug=False,
        num_devices=1 if emulate_collective else NCORES,
    )

    xT = nc.dram_tensor("xT", [K, N], F16, kind="ExternalInput")
    xrT = nc.dram_tensor("xrT", [K, R], F16, kind="ExternalInput")
    adjB = nc.dram_tensor("adjB", [N, R], F16, kind="ExternalInput")
    W_all = nc.dram_tensor("W_all", [K, DALL], F16, kind="ExternalInput")
    wa = nc.dram_tensor("wa", [K, 2 * H], F16, kind="ExternalInput")
    W_out = nc.dram_tensor("W_out", [DALL, D], F16, kind="ExternalInput")
    wa2 = nc.dram_tensor("wa2", [DALL, 2], F16, kind="ExternalInput")
    out = nc.dram_tensor("out", [R, D], F32, kind="ExternalOutput")

    with tile.TileContext(nc) as tc:
        _emit(nc, tc, locals(), emulate_collective)

    nc.compile()
    return nc


def _emit(nc, tc, io, emulate_collective):
    xT, xrT, adjB, W_all, wa, W_out, wa2, out = (
        io["xT"], io["xrT"], io["adjB"], io["W_all"], io["wa"],
        io["W_out"], io["wa2"], io["out"],
    )
    AT = mybir.AluOpType
    AF = mybir.ActivationFunctionType

    from contextlib import ExitStack
    with ExitStack() as ctx:
        res = ctx.enter_context(tc.tile_pool(name="res", bufs=1))
        psum = ctx.enter_context(tc.tile_pool(name="psum", bufs=3, space="PSUM"))
        ppool = ctx.enter_context(tc.tile_pool(name="ppool", bufs=4, space="PSUM"))
        ptr = ctx.enter_context(tc.tile_pool(name="ptr", bufs=1, space="PSUM"))
        work = ctx.enter_context(tc.tile_pool(name="work", bufs=3))
        work2 = ctx.enter_context(tc.tile_pool(name="work2", bufs=3))
        tpool = ctx.enter_context(tc.tile_pool(name="tpool", bufs=6))
        small = ctx.enter_context(tc.tile_pool(name="small", bufs=4))
        rpool = ctx.enter_context(tc.tile_pool(name="rpool", bufs=2))
        dram = ctx.enter_context(tc.tile_pool(name="dram", bufs=1, space="DRAM"))

        # ---- resident SBUF tensors (chunk-major [128, n_chunks*width]) ---- #
        xT_sb = res.tile([128, 4 * N], F16, tag="xT")
        xrT_sb = tpool.tile([128, 4 * R], F16, tag="rep")
        adjB_sb = res.tile([128, JC * R], F16, tag="adjB")     # {inf, 0}
        W_all_sb = res.tile([128, 4 * DALL], F16, tag="W_all")
        wa_sb = res.tile([128, 4 * 2 * H], F16, tag="wa")
        W_out_sb = res.tile([128, 4 * D], F16, tag="W_out")
        wa2_sb = res.tile([128, 4 * 2], F16, tag="wa2")
        whb_sb = res.tile([128, JC * DALL], F16, tag="whb")    # Wh all heads
        fg_sb = res.tile([128, JC * 2 * H], F32, tag="fg")     # f/g raw
        egA_sb = res.tile([128, JC * H], F32, tag="egA")       # exp(g)
        egB_sb = res.tile([128, JC * H], F32, tag="egB")       # exp(.2 g)
        hcat_sb = res.tile([128, 4 * DALL], F16, tag="hcat")   # [i_sub][dall]
        hcatT_sb = res.tile([128, 4 * R], F16, tag="hcatT")    # [dall, R]
        whb2_sb = res.tile([128, JC * AUG2], F16, tag="whb2")  # gathered l2
        g2_sb = res.tile([128, JC], F32, tag="g2")
        eg2A_sb = res.tile([128, JC], F32, tag="eg2A")
        eg2B_sb = res.tile([128, JC], F32, tag="eg2B")
        ones16 = res.tile([128, 1], F16, tag="ones16")
        ident16 = res.tile([128, 128], F16, tag="ident16")
        res2 = res.tile([128, 4 * D], F32, tag="res2")

        def chunked(dram_t, width):
            return dram_t.ap().rearrange("(c p) w -> p c w", p=128)

        def chunked_sb(sb_ap, width):
            return sb_ap.rearrange("p (c w) -> p c w", w=width)

        def load(sb_tile, dram_t, width, split=1, split_free=1):
            dst = chunked_sb(sb_tile[:], width)
            src = chunked(dram_t, width)
            nch = dst.shape[1]
            step = max(1, nch // split)
            fstep = max(1, width // split_free)
            for lo in range(0, nch, step):
                hi = min(nch, lo + step)
                for flo in range(0, width, fstep):
                    fhi = min(width, flo + fstep)
                    nc.sync.dma_start(
                        dst[:, lo:hi, flo:fhi], src[:, lo:hi, flo:fhi])

        # ---- phase 0: loads + constants ---- #
        load(xrT_sb, xrT, R)
        load(wa_sb, wa, 2 * H)
        load(W_all_sb, W_all, DALL)
        load(adjB_sb, adjB, R, split=8)
        load(xT_sb, xT, N, split_free=16)
        load(W_out_sb, W_out, D)
        load(wa2_sb, wa2, 2)
        nc.vector.memset(ones16[:], 1.0)
        masks.make_identity(nc, ident16[:])

        # ---- phase 1 helpers: per-chunk Wh/fg prep ---- #
        prepc_idx = [0]

        def prep_chunk(jc):
            pw = psum.tile([128, DALL], F32, tag="bank")
            pf = psum.tile([128, 2 * H], F32, tag="bank")
            for kc in range(4):
                lhsT = xT_sb[:, kc * N + jc * 128: kc * N + (jc + 1) * 128]
                nc.tensor.matmul(
                    pw[:], lhsT, W_all_sb[:, kc * DALL:(kc + 1) * DALL],
                    start=(kc == 0), stop=(kc == 3),
                )
                nc.tensor.matmul(
                    pf[:], lhsT, wa_sb[:, kc * 2 * H:(kc + 1) * 2 * H],
                    start=(kc == 0), stop=(kc == 3),
                )
            i = prepc_idx[0]
            prepc_idx[0] += 1
            dst = whb_sb[:, jc * DALL:(jc + 1) * DALL]
            if _bres(i, PREPC_POOL, JC):
                nc.gpsimd.tensor_copy(dst, pw[:])
            elif _bres(i, PREPC_ACT, JC):
                nc.scalar.copy(dst, pw[:])
            else:
                nc.vector.tensor_copy(dst, pw[:])
            nc.vector.tensor_copy(fg_sb[:, jc * 2 * H:(jc + 1) * 2 * H], pf[:])

        def group_exps(g):
            # exp(g_h) and exp(.2 g_h) for the 4 chunks of group g, all heads
            src = fg_sb[:].rearrange("p (c h x) -> p c h x", h=H, x=2)
            dA = egA_sb[:].rearrange("p (c h) -> p c h", h=H)
            dB = egB_sb[:].rearrange("p (c h) -> p c h", h=H)
            lo, hi = g * G, (g + 1) * G
            sg = src[:, lo:hi, :, 1:2]
            nc.scalar.activation(dA[:, lo:hi, :].unsqueeze(3), sg, AF.Exp)
            nc.scalar.activation(dB[:, lo:hi, :].unsqueeze(3), sg,
                                 AF.Exp, scale=ALPHA)

        def emit_fg_rows():
            pfr = psum.tile([16, R], F32, tag="bank")
            for kc in range(4):
                nc.tensor.matmul(
                    pfr[:], wa_sb[:, kc * 2 * H:(kc + 1) * 2 * H],
                    xrT_sb[:, kc * R:(kc + 1) * R],
                    start=(kc == 0), stop=(kc == 3),
                )
            rows = res.tile([48, R], F16, tag="fgrows")
            nc.vector.tensor_copy(rows[0:16, :], pfr[:])
            nc.scalar.activation(rows[16:32, :], pfr[:], AF.Exp)
            nc.scalar.activation(rows[32:48, :], pfr[:], AF.Exp, scale=ALPHA)
            fgb_d = dram.tile([48, R], F16, tag="fgb")
            nc.sync.dma_start(fgb_d[:], rows[:])
            return fgb_d

        # ---- attention unit (group-at-a-time emission) ---- #
        def bcast(f_row_dram):
            rep = tpool.tile([128, G * R], F16, tag="rep")
            nc.sync.dma_start(
                rep[:].rearrange("p (c w) -> p c w", w=R),
                f_row_dram.broadcast_to([128, R]).unsqueeze(1)
                .broadcast_to([128, G, R]))
            return rep

        def unit_start(unit, f_rows):
            # f_rows: dict with any of 'f', 'Ef', 'ef' -> [1, R] dram APs
            pout = ppool.tile([128, 4 * 65], F32, tag="pout")
            reps = {k: bcast(v) for k, v in f_rows.items()}
            return pout, reps

        def unit_group(unit, pout, reps, g, lhsT_rhs_of, gcol_of):
            """Emit one group's scores + att matmuls.

            lhsT_rhs_of(jc) -> (whb rhs AP [128, 64 or 65], has_ones)
            gcol_of(kind, jc) -> [128,1] AP: kind in 'g','Eg','eg'
            """
            is_a = _is_A(unit, g)
            pm = work.tile([128, G * R], F16, tag="pm")
            if is_a:
                u = work2.tile([128, G * R], F16, tag="u")
                for c in range(G):
                    jc = g * G + c
                    nc.scalar.activation(
                        u[:, c * R:(c + 1) * R],
                        reps["f"][:, c * R:(c + 1) * R],
                        AF.Prelu, bias=gcol_of("g", jc), alpha=ALPHA)
                q = work2.tile([128, G * R], F16, tag="q")
                nc.scalar.activation(q[:], u[:], AF.Exp)
            else:
                p1 = work2.tile([128, G * R], F16, tag="p1")
                p2 = work2.tile([128, G * R], F16, tag="p2")
                for c in range(G):
                    jc = g * G + c
                    nc.vector.tensor_scalar(
                        p1[:, c * R:(c + 1) * R],
                        reps["Ef"][:, c * R:(c + 1) * R],
                        gcol_of("Eg", jc), None, AT.mult)
                    nc.vector.tensor_scalar(
                        p2[:, c * R:(c + 1) * R],
                        reps["ef"][:, c * R:(c + 1) * R],
                        gcol_of("eg", jc), None, AT.mult)
                q = work2.tile([128, G * R], F16, tag="q")
                eng = nc.gpsimd if _max_on_pool(unit, g) else nc.vector
                eng.tensor_tensor(q[:], p1[:], p2[:], AT.max)
            eng = nc.gpsimd if _mask_on_pool(unit, g) else nc.vector
            eng.tensor_tensor(
                pm[:], q[:], adjB_sb[:, g * G * R:(g + 1) * G * R], AT.min)
            for c in range(G):
                jc = g * G + c
                rhs, has_ones = lhsT_rhs_of(jc)
                w = 65 if has_ones else 64
                for sb in range(4):
                    lhsT = pm[:, c * R + sb * 128: c * R + (sb + 1) * 128]
                    nc.tensor.matmul(
                        pout[:, sb * 65: sb * 65 + w], lhsT, rhs,
                        start=(jc == 0), stop=(jc == JC - 1),
                    )
                    if not has_ones:
                        nc.tensor.matmul(
                            pout[:, sb * 65 + 64: sb * 65 + 65], lhsT,
                            ones16[:],
                            start=(jc == 0), stop=(jc == JC - 1),
                        )

        epi_idx = [0]

        def epilogue(pout, dst_of, dst_f32):
            """dst = elu(att_out / rowsum); dst_of(sb) -> [128, 64] AP."""
            dt = F32 if dst_f32 else F16
            for sb in range(4):
                i = epi_idx[0]
                epi_idx[0] += 1
                pool_t2 = _bres(i, EPI_POOL, 36)
                recip = rpool.tile([128, 1], F32, tag="recip")
                nc.vector.reciprocal(recip[:], pout[:, sb * 65 + 64: sb * 65 + 65])
                hl = small.tile([128, D], dt, tag="ep")
                nc.vector.tensor_scalar(
                    hl[:], pout[:, sb * 65: sb * 65 + 64], recip[:],
                    None, AT.mult)
                # elu(x) = max(x,0) + min(exp(x),1) - 1   (exp monotone)
                q = small.tile([128, D], dt, tag="ep")
                nc.scalar.activation(q[:], hl[:], AF.Exp)
                t1 = small.tile([128, D], dt, tag="ep")
                nc.vector.tensor_scalar(t1[:], q[:], 1.0, -1.0, AT.min, AT.add)
                t2 = small.tile([128, D], dt, tag="ep")
                eng2 = nc.gpsimd if pool_t2 else nc.vector
                eng2.tensor_scalar(t2[:], hl[:], 0.0, None, AT.max)
                eng2.tensor_tensor(dst_of(sb), t1[:], t2[:], AT.add)

        # ---- phases 1+2 interleaved: chunk prep rides along with lead units #
        def l1_args(h):
            def lhsT_rhs_of(jc, h=h):
                return (whb_sb[:, jc * DALL + h * D: jc * DALL + (h + 1) * D],
                        False)

            def gcol_of(kind, jc, h=h):
                if kind == "g":
                    return fg_sb[:, jc * 2 * H + 2 * h + 1:
                                 jc * 2 * H + 2 * h + 2]
                src = egA_sb if kind == "Eg" else egB_sb
                return src[:, jc * H + h: jc * H + h + 1]
            return lhsT_rhs_of, gcol_of

        def l1_rows(fgb_d, h):
            rows = {}
            if any(_is_A(h, g) for g in range(NG)):
                rows["f"] = fgb_d[2 * h: 2 * h + 1, :]
            if not all(_is_A(h, g) for g in range(NG)):
                rows["Ef"] = fgb_d[16 + 2 * h: 16 + 2 * h + 1, :]
                rows["ef"] = fgb_d[32 + 2 * h: 32 + 2 * h + 1, :]
            return rows

        fgb_d = emit_fg_rows()
        NLEAD = 4
        lead = []
        for h in range(NLEAD):
            lhsTr, gco = l1_args(h)
            pout, reps = unit_start(h, l1_rows(fgb_d, h))
            lead.append((h, pout, reps, lhsTr, gco))
        for jc in range(JC):
            prep_chunk(jc)
            if jc % G == G - 1:
                group_exps(jc // G)
                for (h, pout, reps, lhsTr, gco) in lead:
                    unit_group(h, pout, reps, jc // G, lhsTr, gco)

        def l1_dst(h):
            def dst_of(sb, h=h):
                return hcat_sb[:, sb * DALL + h * D: sb * DALL + (h + 1) * D]
            return dst_of

        for h in range(NLEAD):
            epilogue(lead[h][1], l1_dst(h), dst_f32=False)

        for hp in range(NLEAD, H, 2):
            pair = []
            for h in (hp, hp + 1):
                lhsTr, gco = l1_args(h)
                pout, reps = unit_start(h, l1_rows(fgb_d, h))
                pair.append((h, pout, reps, lhsTr, gco))
            for g in range(NG):
                for (h, pout, reps, lhsTr, gco) in pair:
                    unit_group(h, pout, reps, g, lhsTr, gco)
            for (h, pout, reps, lhsTr, gco) in pair:
                epilogue(pout, l1_dst(h), dst_f32=False)

        # ---- hcat -> hcatT via PE transposes ---- #
        hct_idx = [0]
        for sb in range(4):
            for kc in range(4):
                pt = ptr.tile([128, 128], F16, tag="pt")
                nc.tensor.transpose(
                    pt[:], hcat_sb[:, sb * DALL + kc * 128:
                                   sb * DALL + (kc + 1) * 128], ident16[:])
                i = hct_idx[0]
                hct_idx[0] += 1
                dst = hcatT_sb[:, kc * R + sb * 128: kc * R + (sb + 1) * 128]
                if _bres(i, HCT_ACT, 16):
                    nc.scalar.copy(dst, pt[:])
                else:
                    nc.vector.tensor_copy(dst, pt[:])

        # ---- phase 3: layer-2 prep + allgather ---- #
        gt_sb = res.tile([128, 4 * AUG2], F16, tag="gt")
        nc.vector.memset(gt_sb[:], 1.0)   # ones column comes for free
        for ib in range(4):
            pw2 = psum.tile([128, D], F32, tag="bank")
            pg2 = psum.tile([128, 2], F32, tag="bank")
            for kc in range(4):
                lhsT = hcatT_sb[:, kc * R + ib * 128: kc * R + (ib + 1) * 128]
                nc.tensor.matmul(pw2[:], lhsT, W_out_sb[:, kc * D:(kc + 1) * D],
                                 start=(kc == 0), stop=(kc == 3))
                nc.tensor.matmul(pg2[:], lhsT, wa2_sb[:, kc * 2:(kc + 1) * 2],
                                 start=(kc == 0), stop=(kc == 3))
            nc.vector.tensor_copy(gt_sb[:, ib * AUG2: ib * AUG2 + D], pw2[:])
            nc.vector.tensor_copy(
                gt_sb[:, ib * AUG2 + D + 1: ib * AUG2 + D + 2], pg2[:, 1:2])

        pfg2 = psum.tile([2, R], F32, tag="bank")
        for kc in range(4):
            nc.tensor.matmul(pfg2[:], wa2_sb[:, kc * 2:(kc + 1) * 2],
                             hcatT_sb[:, kc * R:(kc + 1) * R],
                             start=(kc == 0), stop=(kc == 3))
        rows2 = res.tile([6, R], F16, tag="fg2rows")
        nc.vector.tensor_copy(rows2[0:2, :], pfg2[:])
        nc.scalar.activation(rows2[2:4, :], pfg2[:], AF.Exp)
        nc.scalar.activation(rows2[4:6, :], pfg2[:], AF.Exp, scale=ALPHA)
        fgb2_d = dram.tile([6, R], F16, tag="fgb2")
        nc.sync.dma_start(fgb2_d[:], rows2[:])

        cc_in = dram.tile([R, AUG2], F16, tag="cc_in")
        cc_space = {} if emulate_collective else {"addr_space": "Shared"}
        cc_out = dram.tile([N, AUG2], F16, tag="cc_out", **cc_space)
        nc.sync.dma_start(
            cc_in[:].rearrange("(c p) w -> p c w", p=128),
            chunked_sb(gt_sb[:], AUG2))
        if emulate_collective:
            for c in range(NCORES):
                nc.sync.dma_start(cc_out[c * R:(c + 1) * R, :], cc_in[:])
        else:
            nc.gpsimd.collective_compute(
                "AllGather", mybir.AluOpType.bypass,
                replica_groups=[list(range(NCORES))],
                ins=[cc_in.opt()], outs=[cc_out.opt()],
            )
        whb2_ch = chunked_sb(whb2_sb[:], AUG2)
        cc_out_ch = cc_out[:].rearrange("(c p) w -> p c w", p=128)
        g2_ch = g2_sb[:].rearrange("p (c w) -> p c w", w=1)
        for half in range(2):
            lo, hi = half * (JC // 2), (half + 1) * (JC // 2)
            nc.sync.dma_start(whb2_ch[:, lo:hi, :], cc_out_ch[:, lo:hi, :])
            nc.vector.tensor_copy(
                g2_ch[:, lo:hi, :], whb2_ch[:, lo:hi, D + 1: D + 2])
        nc.scalar.activation(eg2A_sb[:], g2_sb[:], AF.Exp)
        nc.scalar.activation(eg2B_sb[:], g2_sb[:], AF.Exp, scale=ALPHA)

        # ---- phase 4: layer 2 ---- #
        def l2_lhsT_rhs_of(jc):
            return whb2_sb[:, jc * AUG2: jc * AUG2 + 65], True

        def l2_gcol_of(kind, jc):
            if kind == "g":
                return g2_sb[:, jc: jc + 1]
            src = eg2A_sb if kind == "Eg" else eg2B_sb
            return src[:, jc: jc + 1]

        rows = {}
        if any(_is_A(H, g) for g in range(NG)):
            rows["f"] = fgb2_d[0:1, :]
        if not all(_is_A(H, g) for g in range(NG)):
            rows["Ef"] = fgb2_d[2:3, :]
            rows["ef"] = fgb2_d[4:5, :]
        pout2, reps2 = unit_start(H, rows)
        for g in range(NG):
            unit_group(H, pout2, reps2, g, l2_lhsT_rhs_of, l2_gcol_of)
        epilogue(pout2, lambda sb: res2[:, sb * D:(sb + 1) * D], dst_f32=True)
        nc.sync.dma_start(
            out.ap().rearrange("(c p) w -> p c w", p=128),
            chunked_sb(res2[:], D))


# --------------------------------------------------------------------------- #
# host side
# --------------------------------------------------------------------------- #

def _pack_inputs(x, adj, W_heads, a_src, a_dst, W_out, a_src_out, a_dst_out):
    """Shard + repack the full inputs into the 8 per-core input maps."""
    x = np.asarray(x, np.float32)
    adj = np.asarray(adj)
    W_heads = np.asarray(W_heads, np.float32)
    a_src = np.asarray(a_src, np.float32)
    a_dst = np.asarray(a_dst, np.float32)
    W_out_np = np.asarray(W_out, np.float32)
    a_src_out = np.asarray(a_src_out, np.float32)
    a_dst_out = np.asarray(a_dst_out, np.float32)

    f16 = NPF16
    xT = np.ascontiguousarray(x.T).astype(f16)                       # [K, N]
    W_all = np.ascontiguousarray(
        W_heads.transpose(1, 0, 2).reshape(K, DALL)).astype(f16)     # [K, H*D]
    wa_cols = []
    for h in range(H):
        wa_cols.append(W_heads[h] @ a_src[h])
        wa_cols.append(W_heads[h] @ a_dst[h])
    wa = np.stack(wa_cols, axis=1).astype(f16)                       # [K, 16]
    W_out_p = W_out_np.astype(f16)                                   # [DALL, D]
    wa2 = np.stack([W_out_np @ a_src_out, W_out_np @ a_dst_out],
                   axis=1).astype(f16)                               # [DALL, 2]

    in_maps = []
    for c in range(NCORES):
        rows = slice(c * R, (c + 1) * R)
        adj_rows = np.where(adj[rows, :] > 0, np.inf, 0.0).astype(np.float32)
        adjM = np.ascontiguousarray(adj_rows.T).astype(f16)          # [N, R]
        in_maps.append({
            "xT": xT,
            "xrT": np.ascontiguousarray(x[rows].T).astype(f16),
            "adjB": adjM,
            "W_all": W_all,
            "wa": wa,
            "W_out": W_out_p,
            "wa2": wa2,
        })
    return in_maps


def kernel(**inputs) -> np.ndarray:
    if "nc" not in _CACHE:
        _CACHE["nc"] = _build(emulate_collective=False)
    nc = _CACHE["nc"]
    in_maps = _pack_inputs(**inputs)
    res = run_bass_kernel_spmd(nc, in_maps, core_ids=list(range(NCORES)))
    return np.concatenate([res.results[c]["out"] for c in range(NCORES)], axis=0)


# revision 10
# speedup vs baseline: 1.2815x; 1.2815x over previous
"""Trainium2 Bass kernel for a 2-layer dense GAT (nn_GAT_87144886436203).

Sharding: row-shard the N=4096 nodes across 8 NeuronCores (512 rows each).
Each core computes attention scores for its row block against all N columns,
with the contraction axis j kept on SBUF partitions.

Score pipeline, two interchangeable per-group paths over [128 j, 2048 i]
fp16 tiles:
  A-path (ACT):  u = Prelu(frep + g[j]) per chunk; p = Exp(u) (2048-wide)
  D-path (DVE):  exp(lrelu(f+g)) == max(e^f e^g, e^.2f e^.2g)  (exp monotone)
                 P1 = EfRep * Eg[j]; P2 = efRep * eg[j]  (4x tensor_scalar)
                 q  = max(P1, P2)
Masking uses min(q, adjM) with adjM in {+inf, 0} -- `min` runs in the
GPSIMD 0.6-efficiency class (vs 0.42 for mult), and q > 0 always.

The att matmul runs transposed: out[i_sub 128, 64] += pm[j, i_sub].T @ Wh[j, :]
(PE cost is proportional to OUT free size, so 64-wide beats 512-wide 8x),
with the row-sum accumulated by a second [128,1] ones matmul. The softmax
reciprocal then lands per-partition (cheap tensor_scalar, no broadcast
matmul). hcat is re-transposed to hcatT via 16 PE transposes for layer 2.
One small AllGather moves the per-core layer-2 node features between layers.

Engine assignment (A vs D path, and DVE vs Pool for the max/mask
tensor_tensor ops and prep copies) is tunable via the *_COUNT knobs,
balanced against the TimelineSim cost model.
"""

import numpy as np
import ml_dtypes

import concourse.bass as bass
import concourse.bacc as bacc
import concourse.tile as tile
import concourse.mybir as mybir
from concourse import masks
from concourse.bass_utils import run_bass_kernel_spmd

F16 = mybir.dt.float16
F32 = mybir.dt.float32
NPF16 = ml_dtypes.float16 if hasattr(ml_dtypes, "float16") else np.float16

NCORES = 8
N = 4096            # nodes
K = 512             # input feature dim (= NFEAT)
H = 8               # heads (layer 1)
D = 64              # per-head hidden (= NHID = NCLASS)
DALL = H * D        # 512
R = N // NCORES     # 512 rows per core
JC = N // 128       # 32 j-chunks
G = 4               # j-chunks per group (free dim 2048 for the big ops)
NG = JC // G        # 8 groups
AUG2 = D + 2        # 66: [Wh2 | ones | g2]
ALPHA = 0.2

N_UNITS = H + 1
NSLOTS = N_UNITS * NG     # 72 (unit, group) slots

# ---- engine-assignment knobs (tuned against TimelineSim) ----
A_UNITS = (0, 1, 4, 6)   # ACT-path units, interleaved in time with DVE units
A_L2 = 2           # of the 8 layer-2 groups, how many on the ACT path
MAXP_COUNT = 18    # of the D-slots' max ops, how many go to Pool
MASKP_COUNT = 36   # of the 72 mask ops, how many go to Pool
PREPC_POOL = 4     # of the 32 whb prep copies, how many on Pool
PREPC_ACT = 8      # ... how many on ACT (rest DVE)
EPI_POOL = 10      # of 36 epilogue subtiles, t2/add ops on Pool
HCT_ACT = 8        # of 16 hcatT copies, how many on ACT (rest DVE)


def _bres(i, count, total):
    return (i * count) // total != ((i + 1) * count) // total


def _slot(unit, g):
    return unit * NG + g


def _is_A(unit, g):
    if unit == H:
        return _bres(g, A_L2, NG)
    return unit in A_UNITS


def _max_on_pool(unit, g):
    return _bres(_slot(unit, g), MAXP_COUNT, NSLOTS)


def _mask_on_pool(unit, g):
    return _bres(_slot(unit, g), MASKP_COUNT, NSLOTS)


_CACHE = {}


# --------------------------------------------------------------------------- #
# device program
# --------------------------------------------------------------------------- #

def _build(emulate_collective=False):
    nc = bacc.Bacc(
        "TRN2",
        target_bir_lowering=False,
        debug=False,
        num_devices=1 if emulate_collective else NCORES,
    )

    xT = nc.dram_tensor("xT", [K, N], F16, kind="ExternalInput")
    xrT = nc.dram_tensor("xrT", [K, R], F16, kind="ExternalInput")
    adjB = nc.dram_tensor("adjB", [N, R], F16, kind="ExternalInput")
    W_all = nc.dram_tensor("W_all", [K, DALL], F16, kind="ExternalInput")
    wa = nc.dram_tensor("wa", [K, 2 * H], F16, kind="ExternalInput")
    W_out = nc.dram_tensor("W_out", [DALL, D], F16, kind="ExternalInput")
    wa2 = nc.dram_tensor("wa2", [DALL, 2], F16, kind="ExternalInput")
    out = nc.dram_tensor("out", [R, D], F32, kind="ExternalOutput")

    with tile.TileContext(nc) as tc:
        _emit(nc, tc, locals(), emulate_collective)

    nc.compile()
    return nc


def _emit(nc, tc, io, emulate_collective):
    xT, xrT, adjB, W_all, wa, W_out, wa2, out = (
        io["xT"], io["xrT"], io["adjB"], io["W_all"], io["wa"],
        io["W_out"], io["wa2"], io["out"],
    )
    AT = mybir.AluOpType
    AF = mybir.ActivationFunctionType

    from contextlib import ExitStack
    with ExitStack() as ctx:
        res = ctx.enter_context(tc.tile_pool(name="res", bufs=1))
        psum = ctx.enter_context(tc.tile_pool(name="psum", bufs=3, space="PSUM"))
        ppool = ctx.enter_context(tc.tile_pool(name="ppool", bufs=4, space="PSUM"))
        ptr = ctx.enter_context(tc.tile_pool(name="ptr", bufs=1, space="PSUM"))
        work = ctx.enter_context(tc.tile_pool(name="work", bufs=3))
        work2 = ctx.enter_context(tc.tile_pool(name="work2", bufs=3))
        tpool = ctx.enter_context(tc.tile_pool(name="tpool", bufs=6))
        small = ctx.enter_context(tc.tile_pool(name="small", bufs=4))
        rpool = ctx.enter_context(tc.tile_pool(name="rpool", bufs=2))
        dram = ctx.enter_context(tc.tile_pool(name="dram", bufs=1, space="DRAM"))

        # ---- resident SBUF tensors (chunk-major [128, n_chunks*width]) ---- #
        xT_sb = res.tile([128, 4 * N], F16, tag="xT")
        xrT_sb = tpool.tile([128, 4 * R], F16, tag="rep")
        adjB_sb = res.tile([128, JC * R], F16, tag="adjB")     # {inf, 0}
        W_all_sb = res.tile([128, 4 * DALL], F16, tag="W_all")
        wa_sb = res.tile([128, 4 * 2 * H], F16, tag="wa")
        W_out_sb = res.tile([128, 4 * D], F16, tag="W_out")
        wa2_sb = res.tile([128, 4 * 2], F16, tag="wa2")
        whb_sb = res.tile([128, JC * DALL], F16, tag="whb")    # Wh all heads
        fg_sb = res.tile([128, JC * 2 * H], F32, tag="fg")     # f/g raw
        egA_sb = res.tile([128, JC * H], F32, tag="egA")       # exp(g)
        egB_sb = res.tile([128, JC * H], F32, tag="egB")       # exp(.2 g)
        hcat_sb = res.tile([128, 4 * DALL], F16, tag="hcat")   # [i_sub][dall]
        hcatT_sb = res.tile([128, 4 * R], F16, tag="hcatT")    # [dall, R]
        whb2_sb = res.tile([128, JC * AUG2], F16, tag="whb2")  # gathered l2
        g2_sb = res.tile([128, JC], F32, tag="g2")
        eg2A_sb = res.tile([128, JC], F32, tag="eg2A")
        eg2B_sb = res.tile([128, JC], F32, tag="eg2B")
        ones16 = res.tile([128, 1], F16, tag="ones16")
        ident16 = res.tile([128, 128], F16, tag="ident16")
        res2 = res.tile([128, 4 * D], F32, tag="res2")

        def chunked(dram_t, width):
            return dram_t.ap().rearrange("(c p) w -> p c w", p=128)

        def chunked_sb(sb_ap, width):
            return sb_ap.rearrange("p (c w) -> p c w", w=width)

        def load(sb_tile, dram_t, width, split=1, split_free=1):
            dst = chunked_sb(sb_tile[:], width)
            src = chunked(dram_t, width)
            nch = dst.shape[1]
            step = max(1, nch // split)
            fstep = max(1, width // split_free)
            for lo in range(0, nch, step):
                hi = min(nch, lo + step)
                for flo in range(0, width, fstep):
                    fhi = min(width, flo + fstep)
                    nc.sync.dma_start(
                        dst[:, lo:hi, flo:fhi], src[:, lo:hi, flo:fhi])

        # ---- phase 0: loads + constants ---- #
        load(xrT_sb, xrT, R)
        load(wa_sb, wa, 2 * H)
        load(W_all_sb, W_all, DALL)
        load(adjB_sb, adjB, R, split=8)
        load(xT_sb, xT, N, split_free=16)
        load(W_out_sb, W_out, D)
        load(wa2_sb, wa2, 2)
        nc.vector.memset(ones16[:], 1.0)
        masks.make_identity(nc, ident16[:])

        # ---- phase 1 helpers: per-chunk Wh/fg prep ---- #
        prepc_idx = [0]

        def prep_chunk(jc):
            pw = psum.tile([128, DALL], F32, tag="bank")
            pf = psum.tile([128, 2 * H], F32, tag="bank")
            for kc in range(4):
                lhsT = xT_sb[:, kc * N + jc * 128: kc * N + (jc + 1) * 128]
                nc.tensor.matmul(
                    pw[:], lhsT, W_all_sb[:, kc * DALL:(kc + 1) * DALL],
                    start=(kc == 0), stop=(kc == 3),
                )
                nc.tensor.matmul(
                    pf[:], lhsT, wa_sb[:, kc * 2 * H:(kc + 1) * 2 * H],
                    start=(kc == 0), stop=(kc == 3),
                )
            i = prepc_idx[0]
            prepc_idx[0] += 1
            dst = whb_sb[:, jc * DALL:(jc + 1) * DALL]
            if _bres(i, PREPC_POOL, JC):
                nc.gpsimd.tensor_copy(dst, pw[:])
            elif _bres(i, PREPC_ACT, JC):
                nc.scalar.copy(dst, pw[:])
            else:
                nc.vector.tensor_copy(dst, pw[:])
            nc.vector.tensor_copy(fg_sb[:, jc * 2 * H:(jc + 1) * 2 * H], pf[:])

        def group_exps(g):
            # exp(g_h) and exp(.2 g_h) for the 4 chunks of group g, all heads
            src = fg_sb[:].rearrange("p (c h x) -> p c h x", h=H, x=2)
            dA = egA_sb[:].rearrange("p (c h) -> p c h", h=H)
            dB = egB_sb[:].rearrange("p (c h) -> p c h", h=H)
            lo, hi = g * G, (g + 1) * G
            sg = src[:, lo:hi, :, 1:2]
            nc.scalar.activation(dA[:, lo:hi, :].unsqueeze(3), sg, AF.Exp)
            nc.scalar.activation(dB[:, lo:hi, :].unsqueeze(3), sg,
                                 AF.Exp, scale=ALPHA)

        def emit_fg_rows():
            pfr = psum.tile([16, R], F32, tag="bank")
            for kc in range(4):
                nc.tensor.matmul(
                    pfr[:], wa_sb[:, kc * 2 * H:(kc + 1) * 2 * H],
                    xrT_sb[:, kc * R:(kc + 1) * R],
                    start=(kc == 0), stop=(kc == 3),
                )
            rows = res.tile([48, R], F16, tag="fgrows")
            nc.vector.tensor_copy(rows[0:16, :], pfr[:])
            nc.scalar.activation(rows[16:32, :], pfr[:], AF.Exp)
            nc.scalar.activation(rows[32:48, :], pfr[:], AF.Exp, scale=ALPHA)
            fgb_d = dram.tile([48, R], F16, tag="fgb")
            nc.sync.dma_start(fgb_d[:], rows[:])
            return fgb_d

        # ---- attention unit (group-at-a-time emission) ---- #
        def bcast(f_row_dram):
            rep = tpool.tile([128, G * R], F16, tag="rep")
            nc.sync.dma_start(
                rep[:].rearrange("p (c w) -> p c w", w=R),
                f_row_dram.broadcast_to([128, R]).unsqueeze(1)
                .broadcast_to([128, G, R]))
            return rep

        def unit_start(unit, f_rows):
            # f_rows: dict with any of 'f', 'Ef', 'ef' -> [1, R] dram APs
            pout = ppool.tile([128, 4 * 65], F32, tag="pout")
            reps = {k: bcast(v) for k, v in f_rows.items()}
            return pout, reps

        def unit_group(unit, pout, reps, g, lhsT_rhs_of, gcol_of):
            """Emit one group's scores + att matmuls.

            lhsT_rhs_of(jc) -> (whb rhs AP [128, 64 or 65], has_ones)
            gcol_of(kind, jc) -> [128,1] AP: kind in 'g','Eg','eg'
            """
            is_a = _is_A(unit, g)
            pm = work.tile([128, G * R], F16, tag="pm")
            if is_a:
                u = work2.tile([128, G * R], F16, tag="u")
                for c in range(G):
                    jc = g * G + c
                    nc.scalar.activation(
                        u[:, c * R:(c + 1) * R],
                        reps["f"][:, c * R:(c + 1) * R],
                        AF.Prelu, bias=gcol_of("g", jc), alpha=ALPHA)
                q = work2.tile([128, G * R], F16, tag="q")
                nc.scalar.activation(q[:], u[:], AF.Exp)
            else:
                p1 = work2.tile([128, G * R], F16, tag="p1")
                p2 = work2.tile([128, G * R], F16, tag="p2")
                for c in range(G):
                    jc = g * G + c
                    nc.vector.tensor_scalar(
                        p1[:, c * R:(c + 1) * R],
                        reps["Ef"][:, c * R:(c + 1) * R],
                        gcol_of("Eg", jc), None, AT.mult)
                    nc.vector.tensor_scalar(
                        p2[:, c * R:(c + 1) * R],
                        reps["ef"][:, c * R:(c + 1) * R],
                        gcol_of("eg", jc), None, AT.mult)
                q = work2.tile([128, G * R], F16, tag="q")
                eng = nc.gpsimd if _max_on_pool(unit, g) else nc.vector
                eng.tensor_tensor(q[:], p1[:], p2[:], AT.max)
            eng = nc.gpsimd if _mask_on_pool(unit, g) else nc.vector
            eng.tensor_tensor(
                pm[:], q[:], adjB_sb[:, g * G * R:(g + 1) * G * R], AT.min)
            for c in range(G):
                jc = g * G + c
                rhs, has_ones = lhsT_rhs_of(jc)
                w = 65 if has_ones else 64
                for sb in range(4):
                    lhsT = pm[:, c * R + sb * 128: c * R + (sb + 1) * 128]
                    nc.tensor.matmul(
                        pout[:, sb * 65: sb * 65 + w], lhsT, rhs,
                        start=(jc == 0), stop=(jc == JC - 1),
                    )
                    if not has_ones:
                        nc.tensor.matmul(
                            pout[:, sb * 65 + 64: sb * 65 + 65], lhsT,
                            ones16[:],
                            start=(jc == 0), stop=(jc == JC - 1),
                        )

        epi_idx = [0]

        def epilogue(pout, dst_of, dst_f32):
            """dst = elu(att_out / rowsum); dst_of(sb) -> [128, 64] AP."""
            dt = F32 if dst_f32 else F16
            for sb in range(4):
                i = epi_idx[0]
                epi_idx[0] += 1
                pool_t2 = _bres(i, EPI_POOL, 36)
                recip = rpool.tile([128, 1], F32, tag="recip")
                nc.vector.reciprocal(recip[:], pout[:, sb * 65 + 64: sb * 65 + 65])
                hl = small.tile([128, D], dt, tag="ep")
                nc.vector.tensor_scalar(
                    hl[:], pout[:, sb * 65: sb * 65 + 64], recip[:],
                    None, AT.mult)
                # elu(x) = max(x,0) + min(exp(x),1) - 1   (exp monotone)
                q = small.tile([128, D], dt, tag="ep")
                nc.scalar.activation(q[:], hl[:], AF.Exp)
                t1 = small.tile([128, D], dt, tag="ep")
                nc.vector.tensor_scalar(t1[:], q[:], 1.0, -1.0, AT.min, AT.add)
                t2 = small.tile([128, D], dt, tag="ep")
                eng2 = nc.gpsimd if pool_t2 else nc.vector
                eng2.tensor_scalar(t2[:], hl[:], 0.0, None, AT.max)
                eng2.tensor_tensor(dst_of(sb), t1[:], t2[:], AT.add)

        # ---- phases 1+2 interleaved: chunk prep rides along with lead units #
        def l1_args(h):
            def lhsT_rhs_of(jc, h=h):
                return (whb_sb[:, jc * DALL + h * D: jc * DALL + (h + 1) * D],
                        False)

            def gcol_of(kind, jc, h=h):
                if kind == "g":
                    return fg_sb[:, jc * 2 * H + 2 * h + 1:
                                 jc * 2 * H + 2 * h + 2]
                src = egA_sb if kind == "Eg" else egB_sb
                return src[:, jc * H + h: jc * H + h + 1]
            return lhsT_rhs_of, gcol_of

        def l1_rows(fgb_d, h):
            rows = {}
            if any(_is_A(h, g) for g in range(NG)):
                rows["f"] = fgb_d[2 * h: 2 * h + 1, :]
            if not all(_is_A(h, g) for g in range(NG)):
                rows["Ef"] = fgb_d[16 + 2 * h: 16 + 2 * h + 1, :]
                rows["ef"] = fgb_d[32 + 2 * h: 32 + 2 * h + 1, :]
            return rows

        fgb_d = emit_fg_rows()
        NLEAD = 4
        lead = []
        for h in range(NLEAD):
            lhsTr, gco = l1_args(h)
            pout, reps = unit_start(h, l1_rows(fgb_d, h))
            lead.append((h, pout, reps, lhsTr, gco))
        for jc in range(JC):
            prep_chunk(jc)
            if jc % G == G - 1:
                group_exps(jc // G)
                for (h, pout, reps, lhsTr, gco) in lead:
                    unit_group(h, pout, reps, jc // G, lhsTr, gco)

        def l1_dst(h):
            def dst_of(sb, h=h):
                return hcat_sb[:, sb * DALL + h * D: sb * DALL + (h + 1) * D]
            return dst_of

        for h in range(NLEAD):
            epilogue(lead[h][1], l1_dst(h), dst_f32=False)

        for hp in range(NLEAD, H, 2):
            pair = []
            for h in (hp, hp + 1):
                lhsTr, gco = l1_args(h)
                pout, reps = unit_start(h, l1_rows(fgb_d, h))
                pair.append((h, pout, reps, lhsTr, gco))
            for g in range(NG):
                for (h, pout, reps, lhsTr, gco) in pair:
                    unit_group(h, pout, reps, g, lhsTr, gco)
            for (h, pout, reps, lhsTr, gco) in pair:
                epilogue(pout, l1_dst(h), dst_f32=False)

        # ---- hcat -> hcatT via PE transposes ---- #
        hct_idx = [0]
        for sb in range(4):
            for kc in range(4):
                pt = ptr.tile([128, 128], F16, tag="pt")
                nc.tensor.transpose(
                    pt[:], hcat_sb[:, sb * DALL + kc * 128:
                                   sb * DALL + (kc + 1) * 128], ident16[:])
                i = hct_idx[0]
                hct_idx[0] += 1
                dst = hcatT_sb[:, kc * R + sb * 128: kc * R + (sb + 1) * 128]
                if _bres(i, HCT_ACT, 16):
                    nc.scalar.copy(dst, pt[:])
                else:
                    nc.vector.tensor_copy(dst, pt[:])

        # ---- phase 3: layer-2 prep + allgather ---- #
        gt_sb = res.tile([128, 4 * AUG2], F16, tag="gt")
        nc.vector.memset(gt_sb[:], 1.0)   # ones column comes for free
        for ib in range(4):
            pw2 = psum.tile([128, D], F32, tag="bank")
            pg2 = psum.tile([128, 2], F32, tag="bank")
            for kc in range(4):
                lhsT = hcatT_sb[:, kc * R + ib * 128: kc * R + (ib + 1) * 128]
                nc.tensor.matmul(pw2[:], lhsT, W_out_sb[:, kc * D:(kc + 1) * D],
                                 start=(kc == 0), stop=(kc == 3))
                nc.tensor.matmul(pg2[:], lhsT, wa2_sb[:, kc * 2:(kc + 1) * 2],
                                 start=(kc == 0), stop=(kc == 3))
            nc.vector.tensor_copy(gt_sb[:, ib * AUG2: ib * AUG2 + D], pw2[:])
            nc.vector.tensor_copy(
                gt_sb[:, ib * AUG2 + D + 1: ib * AUG2 + D + 2], pg2[:, 1:2])

        pfg2 = psum.tile([2, R], F32, tag="bank")
        for kc in range(4):
            nc.tensor.matmul(pfg2[:], wa2_sb[:, kc * 2:(kc + 1) * 2],
                             hcatT_sb[:, kc * R:(kc + 1) * R],
                             start=(kc == 0), stop=(kc == 3))
        rows2 = res.tile([6, R], F16, tag="fg2rows")
        nc.vector.tensor_copy(rows2[0:2, :], pfg2[:])
        nc.scalar.activation(rows2[2:4, :], pfg2[:], AF.Exp)
        nc.scalar.activation(rows2[4:6, :], pfg2[:], AF.Exp, scale=ALPHA)
        fgb2_d = dram.tile([6, R], F16, tag="fgb2")
        nc.sync.dma_start(fgb2_d[:], rows2[:])

        cc_in = dram.tile([R, AUG2], F16, tag="cc_in")
        cc_space = {} if emulate_collective else {"addr_space": "Shared"}
        cc_out = dram.tile([N, AUG2], F16, tag="cc_out", **cc_space)
        nc.sync.dma_start(
            cc_in[:].rearrange("(c p) w -> p c w", p=128),
            chunked_sb(gt_sb[:], AUG2))
        if emulate_collective:
            for c in range(NCORES):
                nc.sync.dma_start(cc_out[c * R:(c + 1) * R, :], cc_in[:])
        else:
            nc.gpsimd.collective_compute(
                "AllGather", mybir.AluOpType.bypass,
                replica_groups=[list(range(NCORES))],
                ins=[cc_in.opt()], outs=[cc_out.opt()],
            )
        whb2_ch = chunked_sb(whb2_sb[:], AUG2)
        cc_out_ch = cc_out[:].rearrange("(c p) w -> p c w", p=128)
        g2_ch = g2_sb[:].rearrange("p (c w) -> p c w", w=1)
        for half in range(2):
            lo, hi = half * (JC // 2), (half + 1) * (JC // 2)
            nc.sync.dma_start(whb2_ch[:, lo:hi, :], cc_out_ch[:, lo:hi, :])
            nc.vector.tensor_copy(
                g2_ch[:, lo:hi, :], whb2_ch[:, lo:hi, D + 1: D + 2])
        nc.scalar.activation(eg2A_sb[:], g2_sb[:], AF.Exp)
        nc.scalar.activation(eg2B_sb[:], g2_sb[:], AF.Exp, scale=ALPHA)

        # ---- phase 4: layer 2 ---- #
        def l2_lhsT_rhs_of(jc):
            return whb2_sb[:, jc * AUG2: jc * AUG2 + 65], True

        def l2_gcol_of(kind, jc):
            if kind == "g":
                return g2_sb[:, jc: jc + 1]
            src = eg2A_sb if kind == "Eg" else eg2B_sb
            return src[:, jc: jc + 1]

        rows = {}
        if any(_is_A(H, g) for g in range(NG)):
            rows["f"] = fgb2_d[0:1, :]
        if not all(_is_A(H, g) for g in range(NG)):
            rows["Ef"] = fgb2_d[2:3, :]
            rows["ef"] = fgb2_d[4:5, :]
        pout2, reps2 = unit_start(H, rows)
        for g in range(NG):
            unit_group(H, pout2, reps2, g, l2_lhsT_rhs_of, l2_gcol_of)
        epilogue(pout2, lambda sb: res2[:, sb * D:(sb + 1) * D], dst_f32=True)
        nc.sync.dma_start(
            out.ap().rearrange("(c p) w -> p c w", p=128),
            chunked_sb(res2[:], D))


# --------------------------------------------------------------------------- #
# host side
# --------------------------------------------------------------------------- #

def _pack_inputs(x, adj, W_heads, a_src, a_dst, W_out, a_src_out, a_dst_out):
    """Shard + repack the full inputs into the 8 per-core input maps."""
    x = np.asarray(x, np.float32)
    adj = np.asarray(adj)
    W_heads = np.asarray(W_heads, np.float32)
    a_src = np.asarray(a_src, np.float32)
    a_dst = np.asarray(a_dst, np.float32)
    W_out_np = np.asarray(W_out, np.float32)
    a_src_out = np.asarray(a_src_out, np.float32)
    a_dst_out = np.asarray(a_dst_out, np.float32)

    f16 = NPF16
    xT = np.ascontiguousarray(x.T).astype(f16)                       # [K, N]
    W_all = np.ascontiguousarray(
        W_heads.transpose(1, 0, 2).reshape(K, DALL)).astype(f16)     # [K, H*D]
    wa_cols = []
    for h in range(H):
        wa_cols.append(W_heads[h] @ a_src[h])
        wa_cols.append(W_heads[h] @ a_dst[h])
    wa = np.stack(wa_cols, axis=1).astype(f16)                       # [K, 16]
    W_out_p = W_out_np.astype(f16)                                   # [DALL, D]
    wa2 = np.stack([W_out_np @ a_src_out, W_out_np @ a_dst_out],
                   axis=1).astype(f16)                               # [DALL, 2]

    in_maps = []
    for c in range(NCORES):
        rows = slice(c * R, (c + 1) * R)
        adj_rows = np.where(adj[rows, :] > 0, np.inf, 0.0).astype(np.float32)
        adjM = np.ascontiguousarray(adj_rows.T).astype(f16)          # [N, R]
        in_maps.append({
            "xT": xT,
            "xrT": np.ascontiguousarray(x[rows].T).astype(f16),
            "adjB": adjM,
            "W_all": W_all,
            "wa": wa,
            "W_out": W_out_p,
            "wa2": wa2,
        })
    return in_maps


def kernel(**inputs) -> np.ndarray:
    if "nc" not in _CACHE:
        _CACHE["nc"] = _build(emulate_collective=False)
    nc = _CACHE["nc"]
    in_maps = _pack_inputs(**inputs)
    res = run_bass_kernel_spmd(nc, in_maps, core_ids=list(range(NCORES)))
    return np.concatenate([res.results[c]["out"] for c in range(NCORES)], axis=0)
